# revision 9
# baseline (speedup 1.0000x reference)
"""Trainium2 Bass kernel: greedy bbox-matching loss (nn_BboxLoss).

Full computation: L[t,p] = pairwise bbox loss (IoU / MSE mix), then greedy
per-target argmin over still-available preds, mean of selected losses.

Strategy (8 NeuronCores, preds sharded 8 x 1024, targets replicated):
  device: per core, for each 128-target row tile compute a rank key
          monotone in IoU for every (target, pred) pair and return the
          top-8 preds per target with the pred index PACKED into the low
          mantissa bits (one InstMax, no max_index):
          - corner overlap widths via tensor_scalar min/max (4x DVE mode)
            and PE identity-matmul accumulation into PSUM,
          - per-axis Ln on ACT: key = ln(m1)+ln(m2)-ln(area_p+area_t+eps)
            = ln(inter/S), monotone in IoU; non-overlap (m <= 0) becomes
            NaN which is clamped to -3e38 (junk) on Pool,
          - key assembled by f32r identity matmuls into PSUM, bitwise
            packed with a column iota on DVE, top-8 via one DVE max.
  host:   decode candidates, recompute exact reference-form losses for
          the 64 candidates per target, run the (inherently sequential)
          greedy walk with conservative full-row fallback when the
          candidate lists cannot prove the argmin.
"""
import numpy as np
from contextlib import ExitStack

P_TOTAL = 8192
T = 2048
N_CORES = 8
NP_SHARD = 8                  # pred shards (one per core)
P_CORE = P_TOTAL // NP_SHARD  # 1024 preds per core
NJ = T // 128                 # 16 row tiles of 128 targets
EPS = 1e-7
TOPK = 8
IDX_BITS = 10                 # P_CORE = 1024
IDX_MASK = (1 << IDX_BITS) - 1
KEY_MASK = 0xFFFFFFFF ^ IDX_MASK
CLAMP_VAL = -1.0e30           # junk key for non-overlap (finite: sums stay finite)
INVALID_THR = -1.0e28         # host-side validity threshold on decoded keys
MARGIN = 0.03                 # device-key approximation safety margin

_CACHE = {}


def _build_nc():
    import concourse.bacc as bacc
    import concourse.mybir as mybir
    from concourse.tile import TileContext

    f32 = mybir.dt.float32
    f32r = mybir.dt.float32r
    bf16 = mybir.dt.bfloat16
    u32 = mybir.dt.uint32
    Alu = mybir.AluOpType
    Act = mybir.ActivationFunctionType

    nc = bacc.Bacc()
    # x0p|x1p|y0p|y1p clipped corners then areap, all bf16
    pb_d = nc.dram_tensor("pshard", [1, 5 * P_CORE], bf16, kind="ExternalInput")
    tsc_d = nc.dram_tensor("tscal", [128, 5 * NJ], f32, kind="ExternalInput")
    idb_d = nc.dram_tensor("identb", [128, 256], bf16, kind="ExternalInput")
    idr_d = nc.dram_tensor("identr", [128, 256], f32r, kind="ExternalInput")
    cand_d = nc.dram_tensor("cand", [128, NJ * TOPK], u32, kind="ExternalOutput")

    with TileContext(nc) as tc, ExitStack() as ctx:
        const = ctx.enter_context(tc.tile_pool(name="const", bufs=1))
        work = ctx.enter_context(tc.tile_pool(name="work", bufs=3))
        psA = ctx.enter_context(tc.tile_pool(name="psA", bufs=1, space="PSUM"))
        psB = ctx.enter_context(tc.tile_pool(name="psB", bufs=1, space="PSUM"))
        psF = ctx.enter_context(tc.tile_pool(name="psF", bufs=2, space="PSUM"))

        PL = const.tile([128, 5, P_CORE], bf16)   # corner planes + areap
        TSC = const.tile([128, 5, NJ], f32)
        IDB = const.tile([128, 256], bf16)        # [I | -I] bf16
        IDR = const.tile([128, 256], f32r)        # [I | -I] f32r
        IOTA = const.tile([128, P_CORE], u32)
        MSKC = const.tile([128, 1], u32)
        CAND = const.tile([128, NJ, TOPK], u32)

        nc.sync.dma_start(TSC[:].rearrange("p q j -> p (q j)"), tsc_d[:])
        nc.sync.dma_start(IDB[:], idb_d[:])
        nc.sync.dma_start(IDR[:], idr_d[:])
        # replicate the five per-pred rows across partitions, ordered by use
        PLF = PL[:].rearrange("p q n -> p (q n)")
        for q in range(5):
            nc.sync.dma_start(
                PL[:, q, :],
                pb_d[:, q * P_CORE : (q + 1) * P_CORE].partition_broadcast(128),
            )
        nc.gpsimd.iota(IOTA[:], pattern=[[1, P_CORE]], base=0, channel_multiplier=0)
        nc.vector.memset(MSKC[:], KEY_MASK)

        X0P = PL[:, 0, :]
        X1P = PL[:, 1, :]
        Y0P = PL[:, 2, :]
        Y1P = PL[:, 3, :]
        AREAP = PL[:, 4, :]
        ID_P = IDB[:, 0:128]
        ID_N = IDB[:, 128:256]
        IR_P = IDR[:, 0:128]
        IR_N = IDR[:, 128:256]
        NCH = P_CORE // 512

        for j in range(NJ):
            x0t = TSC[:, 0, j : j + 1]
            x1t = TSC[:, 1, j : j + 1]
            y0t = TSC[:, 2, j : j + 1]
            y1t = TSC[:, 3, j : j + 1]
            ate = TSC[:, 4, j : j + 1]   # area_t + EPS

            CX = work.tile([128, P_CORE], bf16, tag="cx")
            MX = work.tile([128, P_CORE], bf16, tag="mx")
            CY = work.tile([128, P_CORE], bf16, tag="cy")
            MY = work.tile([128, P_CORE], bf16, tag="my")
            LNUV = work.tile([128, 2 * P_CORE], f32, tag="lnuv")
            LUVC = work.tile([128, 2 * P_CORE], f32r, tag="luvc")
            LNS = work.tile([128, P_CORE], f32r, tag="lns")
            PK = work.tile([128, P_CORE], u32, tag="pk")
            M1 = psA.tile([128, P_CORE], f32, tag="m1")
            M2 = psB.tile([128, P_CORE], f32, tag="m2")
            FIN = psF.tile([128, P_CORE], f32, tag="fin")

            # corners: cx = max(x0p, x0t) etc.
            nc.vector.tensor_scalar(CX[:], X0P, x0t, None, op0=Alu.max)
            nc.vector.tensor_scalar(MX[:], X1P, x1t, None, op0=Alu.min)
            nc.vector.tensor_scalar(CY[:], Y0P, y0t, None, op0=Alu.max)
            nc.vector.tensor_scalar(MY[:], Y1P, y1t, None, op0=Alu.min)

            # overlap widths in PSUM: m1 = mx - cx, m2 = my - cy
            for h in range(NCH):
                sl = slice(h * 512, (h + 1) * 512)
                nc.tensor.matmul(M1[:, sl], ID_P, MX[:, sl], start=True, stop=False)
                nc.tensor.matmul(M1[:, sl], ID_N, CX[:, sl], start=False, stop=True)
            for h in range(NCH):
                sl = slice(h * 512, (h + 1) * 512)
                nc.tensor.matmul(M2[:, sl], ID_P, MY[:, sl], start=True, stop=False)
                nc.tensor.matmul(M2[:, sl], ID_N, CY[:, sl], start=False, stop=True)

            # ln of widths; m <= 0 -> NaN, clamped to a finite junk value on
            # Pool BEFORE the fin matmuls (0*NaN = NaN would poison columns)
            nc.scalar.activation(LNUV[:, 0:P_CORE], M1[:], Act.Ln)
            nc.scalar.activation(LNUV[:, P_CORE:], M2[:], Act.Ln)
            nc.gpsimd.tensor_scalar(LUVC[:], LNUV[:], CLAMP_VAL, None, op0=Alu.max)
            nc.scalar.activation(LNS[:], AREAP, Act.Ln, bias=ate)

            # fin = lnu + lnv - lnS  (f32r identity matmuls, PSUM accumulate)
            for h in range(NCH):
                sl = slice(h * 512, (h + 1) * 512)
                sl2 = slice(P_CORE + h * 512, P_CORE + (h + 1) * 512)
                nc.tensor.matmul(FIN[:, sl], IR_P, LUVC[:, sl], start=True, stop=False)
                nc.tensor.matmul(FIN[:, sl], IR_P, LUVC[:, sl2], start=False, stop=False)
                nc.tensor.matmul(FIN[:, sl], IR_N, LNS[:, sl], start=False, stop=True)

            # pack index into low mantissa bits; fin is always finite, so the
            # top-8 max runs directly on the packed values
            nc.vector.scalar_tensor_tensor(
                PK[:], FIN[:].bitcast(u32), MSKC[:, 0:1], IOTA[:],
                op0=Alu.bitwise_and, op1=Alu.bitwise_or,
            )
            nc.vector.max(CAND[:, j, :].bitcast(f32), PK[:].bitcast(f32))

        nc.sync.dma_start(cand_d[:], CAND[:].rearrange("p j k -> p (j k)"))

    nc.compile()
    return nc


def _prep_core_inputs(pred, tgt):
    """Host-side O(P+T) derived quantities. pred [P,4], tgt [T,4] float32."""
    try:
        import ml_dtypes
        bf = ml_dtypes.bfloat16
    except Exception:
        import jax.numpy as jnp
        bf = jnp.bfloat16

    x0t = tgt[:, 0] - tgt[:, 2] / 2
    x1t = tgt[:, 0] + tgt[:, 2] / 2
    y0t = tgt[:, 1] - tgt[:, 3] / 2
    y1t = tgt[:, 1] + tgt[:, 3] / 2
    ate = tgt[:, 2] * tgt[:, 3] + np.float32(EPS)
    tscal = np.stack([x0t, x1t, y0t, y1t, ate]).astype(np.float32)  # [5, T]
    tsc = np.ascontiguousarray(
        tscal.reshape(5, NJ, 128).transpose(2, 0, 1).reshape(128, 5 * NJ)
    )

    ident = np.eye(128, dtype=np.float32)
    idb = np.ascontiguousarray(
        np.concatenate([ident, -ident], axis=1)
    )
    idb_bf = idb.astype(bf)
    idr = idb.astype(np.float32)

    in_maps = []
    for c in range(N_CORES):
        sh = pred[c * P_CORE : (c + 1) * P_CORE]
        x0p = np.maximum(sh[:, 0] - sh[:, 2] / 2, np.float32(0.0))
        x1p = np.minimum(sh[:, 0] + sh[:, 2] / 2, np.float32(1.0))
        y0p = np.maximum(sh[:, 1] - sh[:, 3] / 2, np.float32(0.0))
        y1p = np.minimum(sh[:, 1] + sh[:, 3] / 2, np.float32(1.0))
        areap = sh[:, 2] * sh[:, 3]
        pshard = np.ascontiguousarray(
            np.stack([x0p, x1p, y0p, y1p, areap]).astype(bf).reshape(1, 5 * P_CORE)
        )
        in_maps.append(
            {
                "pshard": pshard,
                "tscal": tsc,
                "identb": idb_bf,
                "identr": idr,
            }
        )
    return in_maps


def _pair_losses(p, t):
    """Reference-form loss for matched pairs p[i] <-> t[i] (numpy f32->f64)."""
    p = p.astype(np.float32); t = t.astype(np.float32)
    x0p = np.maximum(p[:, 0] - p[:, 2] / 2, np.float32(0.0))
    x1p = np.minimum(p[:, 0] + p[:, 2] / 2, np.float32(1.0))
    y0p = np.maximum(p[:, 1] - p[:, 3] / 2, np.float32(0.0))
    y1p = np.minimum(p[:, 1] + p[:, 3] / 2, np.float32(1.0))
    x0t = t[:, 0] - t[:, 2] / 2
    x1t = t[:, 0] + t[:, 2] / 2
    y0t = t[:, 1] - t[:, 3] / 2
    y1t = t[:, 1] + t[:, 3] / 2
    ox0 = np.maximum(x0t, x0p); ox1 = np.minimum(x1t, x1p)
    oy0 = np.maximum(y0t, y0p); oy1 = np.minimum(y1t, y1p)
    nov = (ox1 < ox0) | (oy1 < oy0)
    inter = (ox1 - ox0) * (oy1 - oy0)
    denom = p[:, 2] * p[:, 3] + t[:, 2] * t[:, 3] - inter + np.float32(EPS)
    iou = inter / denom
    mse = np.sum((p - t) * (p - t), axis=-1) / np.float32(4.0)
    return np.where(nov, np.float32(1.0) + mse,
                    np.float32(1.0) - iou).astype(np.float64)


def _row_loss_ref(pred, trow):
    """Reference-form loss of one target row vs all preds (numpy f32)."""
    x0p = np.maximum(pred[:, 0] - pred[:, 2] / 2, np.float32(0.0))
    x1p = np.minimum(pred[:, 0] + pred[:, 2] / 2, np.float32(1.0))
    y0p = np.maximum(pred[:, 1] - pred[:, 3] / 2, np.float32(0.0))
    y1p = np.minimum(pred[:, 1] + pred[:, 3] / 2, np.float32(1.0))
    x0t = trow[0] - trow[2] / 2
    x1t = trow[0] + trow[2] / 2
    y0t = trow[1] - trow[3] / 2
    y1t = trow[1] + trow[3] / 2
    ox0 = np.maximum(x0t, x0p); ox1 = np.minimum(x1t, x1p)
    oy0 = np.maximum(y0t, y0p); oy1 = np.minimum(y1t, y1p)
    nov = (ox1 < ox0) | (oy1 < oy0)
    inter = (ox1 - ox0) * (oy1 - oy0)
    denom = pred[:, 2] * pred[:, 3] + trow[2] * trow[3] - inter + np.float32(EPS)
    iou = inter / denom
    d = pred - trow[None, :]
    mse = np.sum(d * d, axis=-1) / np.float32(4.0)
    return np.where(nov, np.float32(1.0) + mse, np.float32(1.0) - iou)


def _host_greedy(cand_u32, pred, tgt):
    """cand_u32 [N_CORES, T, TOPK]: packed top-8 per (target, pred shard)."""
    NSH = N_CORES
    u = cand_u32.transpose(1, 0, 2).reshape(T, NSH * TOPK)   # [T, 64]
    idx_l = (u & np.uint32(IDX_MASK)).astype(np.int64)
    shard_of = np.broadcast_to(
        np.arange(NSH, dtype=np.int64)[None, :, None], (T, NSH, TOPK)
    ).reshape(T, NSH * TOPK)
    gidx = shard_of * P_CORE + idx_l
    keyf = (u & np.uint32(KEY_MASK)).view(np.float32)
    valid = np.isfinite(keyf) & (keyf > INVALID_THR)

    # exact reference-form loss for every candidate
    tgt_rep = np.repeat(tgt, NSH * TOPK, axis=0)
    loss = _pair_losses(pred[gidx.reshape(-1)], tgt_rep).reshape(T, NSH * TOPK)
    loss[~valid] = np.inf

    nvalid = valid.reshape(T, NSH, TOPK).sum(axis=2)         # [T, NSH]
    order = np.lexsort((gidx, loss), axis=1)                 # per-row

    taken = np.zeros(P_TOTAL, dtype=bool)
    sel = np.empty(T, dtype=np.int64)
    n_fallback = 0
    for t in range(T):
        lt = loss[t]; gt = gidx[t]; ot = order[t]
        chosen = -1
        for d in ot:
            if lt[d] == np.inf:
                break
            if not taken[gt[d]]:
                chosen = d
                break
        safe = chosen >= 0
        if safe:
            closs = lt[chosen]
            # a full shard (8 listed) may hide better preds below its 8th
            # listed key; a partial shard lists ALL its overlap pairs, so it
            # only hides mse-branch pairs (loss >= 1).
            if closs >= np.float32(1.0) - MARGIN:
                safe = False
            else:
                vt = valid[t].reshape(NSH, TOPK)
                ls = lt.reshape(NSH, TOPK)
                gs = gt.reshape(NSH, TOPK)
                for s in range(NSH):
                    if nvalid[t, s] == TOPK:
                        # worst listed candidate of the full shard
                        wl = ls[s, TOPK - 1]
                        if wl < closs + MARGIN and taken[gs[s][vt[s]]].all():
                            safe = False
                            break
        if safe:
            k = gt[chosen]
        else:
            n_fallback += 1
            row = _row_loss_ref(pred, tgt[t]).astype(np.float64)
            row[taken] = np.inf
            k = int(np.argmin(row))
        taken[k] = True
        sel[t] = k
    _host_greedy.n_fallback = n_fallback
    return np.float32(_pair_losses(pred[sel], tgt).mean())


def kernel(pred_bboxes, target_bboxes):
    from concourse.bass_utils import run_bass_kernel_spmd

    pred = np.asarray(pred_bboxes, dtype=np.float32)[0]
    tgt = np.asarray(target_bboxes, dtype=np.float32)[0]

    if "nc" not in _CACHE:
        _CACHE["nc"] = _build_nc()
    nc = _CACHE["nc"]

    in_maps = _prep_core_inputs(pred, tgt)
    res = run_bass_kernel_spmd(nc, in_maps, list(range(N_CORES)))
    cand = _collect(res.results)
    return _host_greedy(cand, pred, tgt)


def _collect(results):
    """results[c]['cand'] [128, NJ*TOPK] u32 -> [N_CORES, T, TOPK]."""
    cand = np.empty((N_CORES, T, TOPK), np.uint32)
    for c in range(N_CORES):
        a = results[c]["cand"].reshape(128, NJ, TOPK)
        # target t = j*128 + p
        cand[c] = a.transpose(1, 0, 2).reshape(T, TOPK)
    return cand


# revision 12
# speedup vs baseline: 1.1589x; 1.1589x over previous
"""Trainium2 Bass kernel: greedy bbox-matching loss (nn_BboxLoss).

Full computation: L[t,p] = pairwise bbox loss (IoU / MSE mix), then greedy
per-target argmin over still-available preds, mean of selected losses.

Strategy (8 NeuronCores, preds sharded 8 x 1024, targets replicated):
  device: per core, for each 128-target row tile compute a rank key
          monotone in IoU for every (target, pred) pair and return the
          top-8 preds per target with the pred index PACKED into the low
          mantissa bits (one InstMax, no max_index):
          - corner overlap widths via tensor_scalar min/max (4x DVE mode)
            and PE identity-matmul accumulation into PSUM,
          - per-axis Ln on ACT: key = ln(m1)+ln(m2)-ln(area_p+area_t+eps)
            = ln(inter/S), monotone in IoU; non-overlap (m <= 0) becomes
            NaN which is clamped to -3e38 (junk) on Pool,
          - key assembled by f32r identity matmuls into PSUM, bitwise
            packed with a column iota on DVE, top-8 via one DVE max.
  host:   decode candidates, recompute exact reference-form losses for
          the 64 candidates per target, run the (inherently sequential)
          greedy walk with conservative full-row fallback when the
          candidate lists cannot prove the argmin.
"""
import numpy as np
from contextlib import ExitStack

P_TOTAL = 8192
T = 2048
N_CORES = 8
NP_SHARD = 8                  # pred shards (one per core)
P_CORE = P_TOTAL // NP_SHARD  # 1024 preds per core
NJ = T // 128                 # 16 row tiles of 128 targets
EPS = 1e-7
TOPK = 8
IDX_BITS = 10                 # P_CORE = 1024
IDX_MASK = (1 << IDX_BITS) - 1
KEY_MASK = 0xFFFFFFFF ^ IDX_MASK
DELTA = 0.02                  # corner clamp: forces widths >= ~DELTA (no NaN keys)
INVALID_THR = -1.0e28         # host-side validity threshold on decoded keys
MARGIN = 0.03                 # device-key approximation safety margin

_CACHE = {}


def _build_nc():
    import concourse.bacc as bacc
    import concourse.mybir as mybir
    from concourse.tile import TileContext

    f32 = mybir.dt.float32
    f32r = mybir.dt.float32r
    bf16 = mybir.dt.bfloat16
    u32 = mybir.dt.uint32
    Alu = mybir.AluOpType
    Act = mybir.ActivationFunctionType

    nc = bacc.Bacc()
    # x0p|x1p|y0p|y1p clipped corners then areap, all bf16
    pb_d = nc.dram_tensor("pshard", [1, 5 * P_CORE], bf16, kind="ExternalInput")
    tsc_d = nc.dram_tensor("tscal", [128, 9 * NJ], f32, kind="ExternalInput")
    idb_d = nc.dram_tensor("identb", [128, 256], bf16, kind="ExternalInput")
    idr_d = nc.dram_tensor("identr", [128, 256], f32r, kind="ExternalInput")
    cand_d = nc.dram_tensor("cand", [128, NJ * TOPK], u32, kind="ExternalOutput")

    with TileContext(nc) as tc, ExitStack() as ctx:
        const = ctx.enter_context(tc.tile_pool(name="const", bufs=1))
        work = ctx.enter_context(tc.tile_pool(name="work", bufs=3))
        psA = ctx.enter_context(tc.tile_pool(name="psA", bufs=1, space="PSUM"))
        psB = ctx.enter_context(tc.tile_pool(name="psB", bufs=1, space="PSUM"))
        psF = ctx.enter_context(tc.tile_pool(name="psF", bufs=2, space="PSUM"))

        PL = const.tile([128, 5, P_CORE], bf16)   # corner planes + areap
        TSC = const.tile([128, 9, NJ], f32)
        IDB = const.tile([128, 256], bf16)        # [I | -I] bf16
        IDR = const.tile([128, 256], f32r)        # [I | -I] f32r
        IOTA = const.tile([128, P_CORE], u32)
        MSKC = const.tile([128, 1], u32)
        CAND = const.tile([128, NJ, TOPK], u32)

        WU = const.tile([128, 1], f32)
        nc.gpsimd.iota(IOTA[:], pattern=[[1, P_CORE]], base=0, channel_multiplier=0)
        nc.vector.memset(MSKC[:], KEY_MASK)
        nc.vector.memset(WU[:], 1.0)
        # warm the Ln activation table while DMAs stream in
        nc.scalar.activation(WU[:], WU[:], Act.Ln)
        nc.sync.dma_start(TSC[:].rearrange("p q j -> p (q j)"), tsc_d[:])
        # replicate the per-pred rows across partitions; x planes first so
        # tile-0 corner work can start as early as possible
        PLF = PL[:].rearrange("p q n -> p (q n)")
        nc.sync.dma_start(
            PLF[:, 0 : 2 * P_CORE],
            pb_d[:, 0 : 2 * P_CORE].partition_broadcast(128),
        )
        nc.sync.dma_start(IDB[:], idb_d[:])
        nc.sync.dma_start(IDR[:], idr_d[:])
        nc.sync.dma_start(
            PLF[:, 2 * P_CORE :],
            pb_d[:, 2 * P_CORE :].partition_broadcast(128),
        )

        X0P = PL[:, 0, :]
        X1P = PL[:, 1, :]
        Y0P = PL[:, 2, :]
        Y1P = PL[:, 3, :]
        AREAP = PL[:, 4, :]
        ID_P = IDB[:, 0:128]
        ID_N = IDB[:, 128:256]
        IR_P = IDR[:, 0:128]
        IR_N = IDR[:, 128:256]
        NCH = P_CORE // 512

        for j in range(NJ):
            x0t = TSC[:, 0, j : j + 1]
            x1t = TSC[:, 1, j : j + 1]
            y0t = TSC[:, 2, j : j + 1]
            y1t = TSC[:, 3, j : j + 1]
            ate = TSC[:, 4, j : j + 1]   # area_t + EPS
            x0d = TSC[:, 5, j : j + 1]   # x0t + DELTA
            x1d = TSC[:, 6, j : j + 1]   # x1t - DELTA
            y0d = TSC[:, 7, j : j + 1]
            y1d = TSC[:, 8, j : j + 1]

            CX = work.tile([128, P_CORE], bf16, tag="cx")
            MX = work.tile([128, P_CORE], bf16, tag="mx")
            CY = work.tile([128, P_CORE], bf16, tag="cy")
            MY = work.tile([128, P_CORE], bf16, tag="my")
            LNUV = work.tile([128, 2 * P_CORE], f32r, tag="lnuv")
            LNS = work.tile([128, P_CORE], f32r, tag="lns")
            PK = work.tile([128, P_CORE], u32, tag="pk")
            M1 = psA.tile([128, P_CORE], f32, tag="m1")
            M2 = psB.tile([128, P_CORE], f32, tag="m2")
            FIN = psF.tile([128, P_CORE], f32, tag="fin")

            # clamped corners: widths mx-cx / my-cy always >= ~DELTA > 0 so
            # Ln never produces NaN/-inf (which would poison the fin matmuls)
            nc.gpsimd.tensor_scalar(CX[:], X0P, x0t, x1d, op0=Alu.max, op1=Alu.min)
            nc.vector.tensor_scalar(MX[:], X1P, x1t, x0d, op0=Alu.min, op1=Alu.max)
            nc.gpsimd.tensor_scalar(CY[:], Y0P, y0t, y1d, op0=Alu.max, op1=Alu.min)
            nc.vector.tensor_scalar(MY[:], Y1P, y1t, y0d, op0=Alu.min, op1=Alu.max)

            # overlap widths in PSUM: m1 = mx - cx, m2 = my - cy
            for h in range(NCH):
                sl = slice(h * 512, (h + 1) * 512)
                nc.tensor.matmul(M1[:, sl], ID_P, MX[:, sl], start=True, stop=False)
                nc.tensor.matmul(M1[:, sl], ID_N, CX[:, sl], start=False, stop=True)
            for h in range(NCH):
                sl = slice(h * 512, (h + 1) * 512)
                nc.tensor.matmul(M2[:, sl], ID_P, MY[:, sl], start=True, stop=False)
                nc.tensor.matmul(M2[:, sl], ID_N, CY[:, sl], start=False, stop=True)

            # ln of (always positive) widths; lnS = ln(areap + ate)
            nc.scalar.activation(LNUV[:, 0:P_CORE], M1[:], Act.Ln)
            nc.scalar.activation(LNUV[:, P_CORE:], M2[:], Act.Ln)
            nc.scalar.activation(LNS[:], AREAP, Act.Ln, bias=ate)

            # fin = lnu + lnv - lnS  (f32r identity matmuls, PSUM accumulate)
            for h in range(NCH):
                sl = slice(h * 512, (h + 1) * 512)
                sl2 = slice(P_CORE + h * 512, P_CORE + (h + 1) * 512)
                nc.tensor.matmul(FIN[:, sl], IR_P, LNUV[:, sl], start=True, stop=False)
                nc.tensor.matmul(FIN[:, sl], IR_P, LNUV[:, sl2], start=False, stop=False)
                nc.tensor.matmul(FIN[:, sl], IR_N, LNS[:, sl], start=False, stop=True)

            # pack index into low mantissa bits; fin is always finite, so the
            # top-8 max runs directly on the packed values
            nc.vector.scalar_tensor_tensor(
                PK[:], FIN[:].bitcast(u32), MSKC[:, 0:1], IOTA[:],
                op0=Alu.bitwise_and, op1=Alu.bitwise_or,
            )
            nc.vector.max(CAND[:, j, :].bitcast(f32), PK[:].bitcast(f32))
            nc.sync.dma_start(
                cand_d[:, j * TOPK : (j + 1) * TOPK], CAND[:, j, :]
            )

    nc.compile()
    return nc


def _prep_core_inputs(pred, tgt):
    """Host-side O(P+T) derived quantities. pred [P,4], tgt [T,4] float32."""
    try:
        import ml_dtypes
        bf = ml_dtypes.bfloat16
    except Exception:
        import jax.numpy as jnp
        bf = jnp.bfloat16

    x0t = tgt[:, 0] - tgt[:, 2] / 2
    x1t = tgt[:, 0] + tgt[:, 2] / 2
    y0t = tgt[:, 1] - tgt[:, 3] / 2
    y1t = tgt[:, 1] + tgt[:, 3] / 2
    ate = tgt[:, 2] * tgt[:, 3] + np.float32(EPS)
    d = np.float32(DELTA)
    tscal = np.stack(
        [x0t, x1t, y0t, y1t, ate, x0t + d, x1t - d, y0t + d, y1t - d]
    ).astype(np.float32)  # [9, T]
    tsc = np.ascontiguousarray(
        tscal.reshape(9, NJ, 128).transpose(2, 0, 1).reshape(128, 9 * NJ)
    )

    ident = np.eye(128, dtype=np.float32)
    idb = np.ascontiguousarray(
        np.concatenate([ident, -ident], axis=1)
    )
    idb_bf = idb.astype(bf)
    idr = idb.astype(np.float32)

    in_maps = []
    for c in range(N_CORES):
        sh = pred[c * P_CORE : (c + 1) * P_CORE]
        x0p = np.maximum(sh[:, 0] - sh[:, 2] / 2, np.float32(0.0))
        x1p = np.minimum(sh[:, 0] + sh[:, 2] / 2, np.float32(1.0))
        y0p = np.maximum(sh[:, 1] - sh[:, 3] / 2, np.float32(0.0))
        y1p = np.minimum(sh[:, 1] + sh[:, 3] / 2, np.float32(1.0))
        areap = sh[:, 2] * sh[:, 3]
        pshard = np.ascontiguousarray(
            np.stack([x0p, x1p, y0p, y1p, areap]).astype(bf).reshape(1, 5 * P_CORE)
        )
        in_maps.append(
            {
                "pshard": pshard,
                "tscal": tsc,
                "identb": idb_bf,
                "identr": idr,
            }
        )
    return in_maps


def _pair_losses(p, t):
    """Reference-form loss for matched pairs p[i] <-> t[i] (numpy f32->f64)."""
    p = p.astype(np.float32); t = t.astype(np.float32)
    x0p = np.maximum(p[:, 0] - p[:, 2] / 2, np.float32(0.0))
    x1p = np.minimum(p[:, 0] + p[:, 2] / 2, np.float32(1.0))
    y0p = np.maximum(p[:, 1] - p[:, 3] / 2, np.float32(0.0))
    y1p = np.minimum(p[:, 1] + p[:, 3] / 2, np.float32(1.0))
    x0t = t[:, 0] - t[:, 2] / 2
    x1t = t[:, 0] + t[:, 2] / 2
    y0t = t[:, 1] - t[:, 3] / 2
    y1t = t[:, 1] + t[:, 3] / 2
    ox0 = np.maximum(x0t, x0p); ox1 = np.minimum(x1t, x1p)
    oy0 = np.maximum(y0t, y0p); oy1 = np.minimum(y1t, y1p)
    nov = (ox1 < ox0) | (oy1 < oy0)
    inter = (ox1 - ox0) * (oy1 - oy0)
    denom = p[:, 2] * p[:, 3] + t[:, 2] * t[:, 3] - inter + np.float32(EPS)
    iou = inter / denom
    mse = np.sum((p - t) * (p - t), axis=-1) / np.float32(4.0)
    return np.where(nov, np.float32(1.0) + mse,
                    np.float32(1.0) - iou).astype(np.float64)


def _row_loss_ref(pred, trow):
    """Reference-form loss of one target row vs all preds (numpy f32)."""
    x0p = np.maximum(pred[:, 0] - pred[:, 2] / 2, np.float32(0.0))
    x1p = np.minimum(pred[:, 0] + pred[:, 2] / 2, np.float32(1.0))
    y0p = np.maximum(pred[:, 1] - pred[:, 3] / 2, np.float32(0.0))
    y1p = np.minimum(pred[:, 1] + pred[:, 3] / 2, np.float32(1.0))
    x0t = trow[0] - trow[2] / 2
    x1t = trow[0] + trow[2] / 2
    y0t = trow[1] - trow[3] / 2
    y1t = trow[1] + trow[3] / 2
    ox0 = np.maximum(x0t, x0p); ox1 = np.minimum(x1t, x1p)
    oy0 = np.maximum(y0t, y0p); oy1 = np.minimum(y1t, y1p)
    nov = (ox1 < ox0) | (oy1 < oy0)
    inter = (ox1 - ox0) * (oy1 - oy0)
    denom = pred[:, 2] * pred[:, 3] + trow[2] * trow[3] - inter + np.float32(EPS)
    iou = inter / denom
    d = pred - trow[None, :]
    mse = np.sum(d * d, axis=-1) / np.float32(4.0)
    return np.where(nov, np.float32(1.0) + mse, np.float32(1.0) - iou)


def _host_greedy(cand_u32, pred, tgt):
    """cand_u32 [N_CORES, T, TOPK]: packed top-8 per (target, pred shard)."""
    NSH = N_CORES
    u = cand_u32.transpose(1, 0, 2).reshape(T, NSH * TOPK)   # [T, 64]
    idx_l = (u & np.uint32(IDX_MASK)).astype(np.int64)
    shard_of = np.broadcast_to(
        np.arange(NSH, dtype=np.int64)[None, :, None], (T, NSH, TOPK)
    ).reshape(T, NSH * TOPK)
    gidx = shard_of * P_CORE + idx_l
    keyf = (u & np.uint32(KEY_MASK)).view(np.float32)
    valid = np.isfinite(keyf) & (keyf > INVALID_THR)

    # exact reference-form loss for every candidate
    tgt_rep = np.repeat(tgt, NSH * TOPK, axis=0)
    loss = _pair_losses(pred[gidx.reshape(-1)], tgt_rep).reshape(T, NSH * TOPK)
    loss[~valid] = np.inf

    order = np.lexsort((gidx, loss), axis=1)                 # per-row

    # sound hidden-candidate bound per (row, shard): every unlisted pair has
    # (possibly delta-boosted) device key <= the 8th listed key, and boosting
    # only raises keys, so its true iou <= iou(key8) and its true loss
    # >= 1 - iou(key8).  key8 decodes >= the stored key (mask clears low
    # mantissa bits of a negative float), keeping the bound conservative.
    key8 = keyf.reshape(T, NSH, TOPK)[:, :, TOPK - 1].astype(np.float64)
    g8 = np.exp(np.minimum(key8, -1e-12))
    hidden_bound = 1.0 - g8 / (1.0 - g8)                     # [T, NSH]
    hidden_bound_min = hidden_bound.min(axis=1)              # [T]

    taken = np.zeros(P_TOTAL, dtype=bool)
    sel = np.empty(T, dtype=np.int64)
    n_fallback = 0
    for t in range(T):
        lt = loss[t]; gt = gidx[t]; ot = order[t]
        chosen = -1
        for d in ot:
            if lt[d] == np.inf:
                break
            if not taken[gt[d]]:
                chosen = d
                break
        safe = chosen >= 0
        if safe:
            closs = lt[chosen]
            # hidden mse-branch pairs have loss >= 1; hidden overlap pairs
            # are bounded by the per-shard key8 bound
            if closs >= np.float32(1.0) - MARGIN:
                safe = False
            elif hidden_bound_min[t] < closs + MARGIN:
                safe = False
        if safe:
            k = gt[chosen]
        else:
            n_fallback += 1
            row = _row_loss_ref(pred, tgt[t]).astype(np.float64)
            row[taken] = np.inf
            k = int(np.argmin(row))
        taken[k] = True
        sel[t] = k
    _host_greedy.n_fallback = n_fallback
    return np.float32(_pair_losses(pred[sel], tgt).mean())


def kernel(pred_bboxes, target_bboxes):
    from concourse.bass_utils import run_bass_kernel_spmd

    pred = np.asarray(pred_bboxes, dtype=np.float32)[0]
    tgt = np.asarray(target_bboxes, dtype=np.float32)[0]

    if "nc" not in _CACHE:
        _CACHE["nc"] = _build_nc()
    nc = _CACHE["nc"]

    in_maps = _prep_core_inputs(pred, tgt)
    res = run_bass_kernel_spmd(nc, in_maps, list(range(N_CORES)))
    cand = _collect(res.results)
    return _host_greedy(cand, pred, tgt)


def _collect(results):
    """results[c]['cand'] [128, NJ*TOPK] u32 -> [N_CORES, T, TOPK]."""
    cand = np.empty((N_CORES, T, TOPK), np.uint32)
    for c in range(N_CORES):
        a = results[c]["cand"].reshape(128, NJ, TOPK)
        # target t = j*128 + p
        cand[c] = a.transpose(1, 0, 2).reshape(T, TOPK)
    return cand


# revision 13
# speedup vs baseline: 1.2452x; 1.0744x over previous
"""Trainium2 Bass kernel: greedy bbox-matching loss (nn_BboxLoss).

Full computation: L[t,p] = pairwise bbox loss (IoU / MSE mix), then greedy
per-target argmin over still-available preds, mean of selected losses.

Strategy (8 NeuronCores, preds sharded 8 x 1024, targets replicated):
  device: per core, for each 128-target row tile compute a rank key
          monotone in IoU for every (target, pred) pair and return the
          top-8 preds per target with the pred index PACKED into the low
          mantissa bits (one InstMax, no max_index):
          - corner overlap widths via tensor_scalar min/max (4x DVE mode)
            and PE identity-matmul accumulation into PSUM,
          - per-axis Ln on ACT: key = ln(m1)+ln(m2)-ln(area_p+area_t+eps)
            = ln(inter/S), monotone in IoU; non-overlap (m <= 0) becomes
            NaN which is clamped to -3e38 (junk) on Pool,
          - key assembled by f32r identity matmuls into PSUM, bitwise
            packed with a column iota on DVE, top-8 via one DVE max.
  host:   decode candidates, recompute exact reference-form losses for
          the 64 candidates per target, run the (inherently sequential)
          greedy walk with conservative full-row fallback when the
          candidate lists cannot prove the argmin.
"""
import numpy as np
from contextlib import ExitStack

P_TOTAL = 8192
T = 2048
N_CORES = 8
NP_SHARD = 8                  # pred shards (one per core)
P_CORE = P_TOTAL // NP_SHARD  # 1024 preds per core
NJ = T // 128                 # 16 row tiles of 128 targets
EPS = 1e-7
TOPK = 8
IDX_BITS = 10                 # P_CORE = 1024
IDX_MASK = (1 << IDX_BITS) - 1
KEY_MASK = 0xFFFFFFFF ^ IDX_MASK
DELTA = 0.02                  # corner clamp: forces widths >= ~DELTA (no NaN keys)
INVALID_THR = -1.0e28         # host-side validity threshold on decoded keys
MARGIN = 0.03                 # device-key approximation safety margin

_CACHE = {}


def _build_nc():
    import concourse.bacc as bacc
    import concourse.mybir as mybir
    from concourse.tile import TileContext

    f32 = mybir.dt.float32
    f32r = mybir.dt.float32r
    bf16 = mybir.dt.bfloat16
    u32 = mybir.dt.uint32
    Alu = mybir.AluOpType
    Act = mybir.ActivationFunctionType

    nc = bacc.Bacc()
    # x0p|x1p|y0p|y1p clipped corners then areap, all bf16
    pb_d = nc.dram_tensor("pshard", [1, 5 * P_CORE], bf16, kind="ExternalInput")
    tsc_d = nc.dram_tensor("tscal", [128, 9 * NJ], f32, kind="ExternalInput")
    idb_d = nc.dram_tensor("identb", [128, 256], bf16, kind="ExternalInput")
    idr_d = nc.dram_tensor("identr", [128, 256], f32r, kind="ExternalInput")
    iota_d = nc.dram_tensor("iotain", [1, P_CORE], u32, kind="ExternalInput")
    cand_d = nc.dram_tensor("cand", [128, NJ * TOPK], u32, kind="ExternalOutput")

    with TileContext(nc) as tc, ExitStack() as ctx:
        const = ctx.enter_context(tc.tile_pool(name="const", bufs=1))
        work = ctx.enter_context(tc.tile_pool(name="work", bufs=3))
        psA = ctx.enter_context(tc.tile_pool(name="psA", bufs=1, space="PSUM"))
        psB = ctx.enter_context(tc.tile_pool(name="psB", bufs=1, space="PSUM"))
        psF = ctx.enter_context(tc.tile_pool(name="psF", bufs=2, space="PSUM"))

        PL = const.tile([128, 5, P_CORE], bf16)   # corner planes + areap
        TSC = const.tile([128, 9, NJ], f32)
        IDB = const.tile([128, 256], bf16)        # [I | -I] bf16
        IDR = const.tile([128, 256], f32r)        # [I | -I] f32r
        IOTA = const.tile([128, P_CORE], u32)
        MSKC = const.tile([128, 1], u32)
        CAND = const.tile([128, NJ, TOPK], u32)

        WU = const.tile([128, 1], f32)
        nc.vector.memset(MSKC[:], KEY_MASK)
        nc.vector.memset(WU[:], 1.0)
        # warm the Ln activation table while DMAs stream in
        nc.scalar.activation(WU[:], WU[:], Act.Ln)
        nc.sync.dma_start(TSC[:].rearrange("p q j -> p (q j)"), tsc_d[:])
        # replicate the per-pred rows across partitions; x planes first so
        # tile-0 corner work can start as early as possible
        PLF = PL[:].rearrange("p q n -> p (q n)")
        nc.sync.dma_start(
            PLF[:, 0 : 2 * P_CORE],
            pb_d[:, 0 : 2 * P_CORE].partition_broadcast(128),
        )
        nc.sync.dma_start(IDB[:], idb_d[:])
        nc.sync.dma_start(IDR[:], idr_d[:])
        nc.sync.dma_start(
            PLF[:, 2 * P_CORE :],
            pb_d[:, 2 * P_CORE :].partition_broadcast(128),
        )
        nc.sync.dma_start(IOTA[:], iota_d[:].partition_broadcast(128))

        X0P = PL[:, 0, :]
        X1P = PL[:, 1, :]
        Y0P = PL[:, 2, :]
        Y1P = PL[:, 3, :]
        AREAP = PL[:, 4, :]
        ID_P = IDB[:, 0:128]
        ID_N = IDB[:, 128:256]
        IR_P = IDR[:, 0:128]
        IR_N = IDR[:, 128:256]
        NCH = P_CORE // 512

        def back_half(j, FIN, LNUV, LNS):
            PK = work.tile([128, P_CORE], u32, tag="pk")
            for h in range(NCH):
                sl = slice(h * 512, (h + 1) * 512)
                sl2 = slice(P_CORE + h * 512, P_CORE + (h + 1) * 512)
                nc.tensor.matmul(FIN[:, sl], IR_P, LNUV[:, sl], start=True, stop=False)
                nc.tensor.matmul(FIN[:, sl], IR_P, LNUV[:, sl2], start=False, stop=False)
                nc.tensor.matmul(FIN[:, sl], IR_N, LNS[:, sl], start=False, stop=True)
            nc.vector.scalar_tensor_tensor(
                PK[:], FIN[:].bitcast(u32), MSKC[:, 0:1], IOTA[:],
                op0=Alu.bitwise_and, op1=Alu.bitwise_or,
            )
            nc.vector.max(CAND[:, j, :].bitcast(f32), PK[:].bitcast(f32))
            nc.sync.dma_start(
                cand_d[:, j * TOPK : (j + 1) * TOPK], CAND[:, j, :]
            )

        pending = None
        for j in range(NJ):
            x0t = TSC[:, 0, j : j + 1]
            x1t = TSC[:, 1, j : j + 1]
            y0t = TSC[:, 2, j : j + 1]
            y1t = TSC[:, 3, j : j + 1]
            ate = TSC[:, 4, j : j + 1]   # area_t + EPS
            x0d = TSC[:, 5, j : j + 1]   # x0t + DELTA
            x1d = TSC[:, 6, j : j + 1]   # x1t - DELTA
            y0d = TSC[:, 7, j : j + 1]
            y1d = TSC[:, 8, j : j + 1]

            CX = work.tile([128, P_CORE], bf16, tag="cx")
            MX = work.tile([128, P_CORE], bf16, tag="mx")
            CY = work.tile([128, P_CORE], bf16, tag="cy")
            MY = work.tile([128, P_CORE], bf16, tag="my")
            LNUV = work.tile([128, 2 * P_CORE], f32r, tag="lnuv")
            LNS = work.tile([128, P_CORE], f32r, tag="lns")
            M1 = psA.tile([128, P_CORE], f32, tag="m1")
            M2 = psB.tile([128, P_CORE], f32, tag="m2")
            FIN = psF.tile([128, P_CORE], f32, tag="fin")

            # clamped corners: widths mx-cx / my-cy always >= ~DELTA > 0 so
            # Ln never produces NaN/-inf (which would poison the fin matmuls)
            nc.gpsimd.tensor_scalar(CX[:], X0P, x0t, x1d, op0=Alu.max, op1=Alu.min)
            nc.vector.tensor_scalar(MX[:], X1P, x1t, x0d, op0=Alu.min, op1=Alu.max)
            nc.gpsimd.tensor_scalar(CY[:], Y0P, y0t, y1d, op0=Alu.max, op1=Alu.min)
            nc.vector.tensor_scalar(MY[:], Y1P, y1t, y0d, op0=Alu.min, op1=Alu.max)

            # overlap widths in PSUM: m1 = mx - cx, m2 = my - cy
            for h in range(NCH):
                sl = slice(h * 512, (h + 1) * 512)
                nc.tensor.matmul(M1[:, sl], ID_P, MX[:, sl], start=True, stop=False)
                nc.tensor.matmul(M1[:, sl], ID_N, CX[:, sl], start=False, stop=True)
            for h in range(NCH):
                sl = slice(h * 512, (h + 1) * 512)
                nc.tensor.matmul(M2[:, sl], ID_P, MY[:, sl], start=True, stop=False)
                nc.tensor.matmul(M2[:, sl], ID_N, CY[:, sl], start=False, stop=True)

            # ln of (always positive) widths; lnS = ln(areap + ate)
            nc.scalar.activation(LNS[:], AREAP, Act.Ln, bias=ate)
            nc.scalar.activation(LNUV[:, 0:P_CORE], M1[:], Act.Ln)
            nc.scalar.activation(LNUV[:, P_CORE:], M2[:], Act.Ln)

            # deferred back half of the previous tile: fin matmuls after this
            # tile's m-matmuls keeps the PE stream bubble-free
            if pending is not None:
                back_half(*pending)
            pending = (j, FIN, LNUV, LNS)
        back_half(*pending)

    nc.compile()
    return nc


def _prep_core_inputs(pred, tgt):
    """Host-side O(P+T) derived quantities. pred [P,4], tgt [T,4] float32."""
    try:
        import ml_dtypes
        bf = ml_dtypes.bfloat16
    except Exception:
        import jax.numpy as jnp
        bf = jnp.bfloat16

    x0t = tgt[:, 0] - tgt[:, 2] / 2
    x1t = tgt[:, 0] + tgt[:, 2] / 2
    y0t = tgt[:, 1] - tgt[:, 3] / 2
    y1t = tgt[:, 1] + tgt[:, 3] / 2
    ate = tgt[:, 2] * tgt[:, 3] + np.float32(EPS)
    d = np.float32(DELTA)
    tscal = np.stack(
        [x0t, x1t, y0t, y1t, ate, x0t + d, x1t - d, y0t + d, y1t - d]
    ).astype(np.float32)  # [9, T]
    tsc = np.ascontiguousarray(
        tscal.reshape(9, NJ, 128).transpose(2, 0, 1).reshape(128, 9 * NJ)
    )

    ident = np.eye(128, dtype=np.float32)
    idb = np.ascontiguousarray(
        np.concatenate([ident, -ident], axis=1)
    )
    idb_bf = idb.astype(bf)
    idr = idb.astype(np.float32)

    in_maps = []
    for c in range(N_CORES):
        sh = pred[c * P_CORE : (c + 1) * P_CORE]
        x0p = np.maximum(sh[:, 0] - sh[:, 2] / 2, np.float32(0.0))
        x1p = np.minimum(sh[:, 0] + sh[:, 2] / 2, np.float32(1.0))
        y0p = np.maximum(sh[:, 1] - sh[:, 3] / 2, np.float32(0.0))
        y1p = np.minimum(sh[:, 1] + sh[:, 3] / 2, np.float32(1.0))
        areap = sh[:, 2] * sh[:, 3]
        pshard = np.ascontiguousarray(
            np.stack([x0p, x1p, y0p, y1p, areap]).astype(bf).reshape(1, 5 * P_CORE)
        )
        in_maps.append(
            {
                "pshard": pshard,
                "tscal": tsc,
                "identb": idb_bf,
                "identr": idr,
            }
        )
    return in_maps


def _pair_losses(p, t):
    """Reference-form loss for matched pairs p[i] <-> t[i] (numpy f32->f64)."""
    p = p.astype(np.float32); t = t.astype(np.float32)
    x0p = np.maximum(p[:, 0] - p[:, 2] / 2, np.float32(0.0))
    x1p = np.minimum(p[:, 0] + p[:, 2] / 2, np.float32(1.0))
    y0p = np.maximum(p[:, 1] - p[:, 3] / 2, np.float32(0.0))
    y1p = np.minimum(p[:, 1] + p[:, 3] / 2, np.float32(1.0))
    x0t = t[:, 0] - t[:, 2] / 2
    x1t = t[:, 0] + t[:, 2] / 2
    y0t = t[:, 1] - t[:, 3] / 2
    y1t = t[:, 1] + t[:, 3] / 2
    ox0 = np.maximum(x0t, x0p); ox1 = np.minimum(x1t, x1p)
    oy0 = np.maximum(y0t, y0p); oy1 = np.minimum(y1t, y1p)
    nov = (ox1 < ox0) | (oy1 < oy0)
    inter = (ox1 - ox0) * (oy1 - oy0)
    denom = p[:, 2] * p[:, 3] + t[:, 2] * t[:, 3] - inter + np.float32(EPS)
    iou = inter / denom
    mse = np.sum((p - t) * (p - t), axis=-1) / np.float32(4.0)
    return np.where(nov, np.float32(1.0) + mse,
                    np.float32(1.0) - iou).astype(np.float64)


def _row_loss_ref(pred, trow):
    """Reference-form loss of one target row vs all preds (numpy f32)."""
    x0p = np.maximum(pred[:, 0] - pred[:, 2] / 2, np.float32(0.0))
    x1p = np.minimum(pred[:, 0] + pred[:, 2] / 2, np.float32(1.0))
    y0p = np.maximum(pred[:, 1] - pred[:, 3] / 2, np.float32(0.0))
    y1p = np.minimum(pred[:, 1] + pred[:, 3] / 2, np.float32(1.0))
    x0t = trow[0] - trow[2] / 2
    x1t = trow[0] + trow[2] / 2
    y0t = trow[1] - trow[3] / 2
    y1t = trow[1] + trow[3] / 2
    ox0 = np.maximum(x0t, x0p); ox1 = np.minimum(x1t, x1p)
    oy0 = np.maximum(y0t, y0p); oy1 = np.minimum(y1t, y1p)
    nov = (ox1 < ox0) | (oy1 < oy0)
    inter = (ox1 - ox0) * (oy1 - oy0)
    denom = pred[:, 2] * pred[:, 3] + trow[2] * trow[3] - inter + np.float32(EPS)
    iou = inter / denom
    d = pred - trow[None, :]
    mse = np.sum(d * d, axis=-1) / np.float32(4.0)
    return np.where(nov, np.float32(1.0) + mse, np.float32(1.0) - iou)


def _host_greedy(cand_u32, pred, tgt):
    """cand_u32 [N_CORES, T, TOPK]: packed top-8 per (target, pred shard)."""
    NSH = N_CORES
    u = cand_u32.transpose(1, 0, 2).reshape(T, NSH * TOPK)   # [T, 64]
    idx_l = (u & np.uint32(IDX_MASK)).astype(np.int64)
    shard_of = np.broadcast_to(
        np.arange(NSH, dtype=np.int64)[None, :, None], (T, NSH, TOPK)
    ).reshape(T, NSH * TOPK)
    gidx = shard_of * P_CORE + idx_l
    keyf = (u & np.uint32(KEY_MASK)).view(np.float32)
    valid = np.isfinite(keyf) & (keyf > INVALID_THR)

    # exact reference-form loss for every candidate
    tgt_rep = np.repeat(tgt, NSH * TOPK, axis=0)
    loss = _pair_losses(pred[gidx.reshape(-1)], tgt_rep).reshape(T, NSH * TOPK)
    loss[~valid] = np.inf

    order = np.lexsort((gidx, loss), axis=1)                 # per-row

    # sound hidden-candidate bound per (row, shard): every unlisted pair has
    # (possibly delta-boosted) device key <= the 8th listed key, and boosting
    # only raises keys, so its true iou <= iou(key8) and its true loss
    # >= 1 - iou(key8).  key8 decodes >= the stored key (mask clears low
    # mantissa bits of a negative float), keeping the bound conservative.
    key8 = keyf.reshape(T, NSH, TOPK)[:, :, TOPK - 1].astype(np.float64)
    g8 = np.exp(np.minimum(key8, -1e-12))
    hidden_bound = 1.0 - g8 / (1.0 - g8)                     # [T, NSH]
    hidden_bound_min = hidden_bound.min(axis=1)              # [T]

    taken = np.zeros(P_TOTAL, dtype=bool)
    sel = np.empty(T, dtype=np.int64)
    n_fallback = 0
    for t in range(T):
        lt = loss[t]; gt = gidx[t]; ot = order[t]
        chosen = -1
        for d in ot:
            if lt[d] == np.inf:
                break
            if not taken[gt[d]]:
                chosen = d
                break
        safe = chosen >= 0
        if safe:
            closs = lt[chosen]
            # hidden mse-branch pairs have loss >= 1; hidden overlap pairs
            # are bounded by the per-shard key8 bound
            if closs >= np.float32(1.0) - MARGIN:
                safe = False
            elif hidden_bound_min[t] < closs + MARGIN:
                safe = False
        if safe:
            k = gt[chosen]
        else:
            n_fallback += 1
            row = _row_loss_ref(pred, tgt[t]).astype(np.float64)
            row[taken] = np.inf
            k = int(np.argmin(row))
        taken[k] = True
        sel[t] = k
    _host_greedy.n_fallback = n_fallback
    return np.float32(_pair_losses(pred[sel], tgt).mean())


def kernel(pred_bboxes, target_bboxes):
    from concourse.bass_utils import run_bass_kernel_spmd

    pred = np.asarray(pred_bboxes, dtype=np.float32)[0]
    tgt = np.asarray(target_bboxes, dtype=np.float32)[0]

    if "nc" not in _CACHE:
        _CACHE["nc"] = _build_nc()
    nc = _CACHE["nc"]

    in_maps = _prep_core_inputs(pred, tgt)
    res = run_bass_kernel_spmd(nc, in_maps, list(range(N_CORES)))
    cand = _collect(res.results)
    return _host_greedy(cand, pred, tgt)


def _collect(results):
    """results[c]['cand'] [128, NJ*TOPK] u32 -> [N_CORES, T, TOPK]."""
    cand = np.empty((N_CORES, T, TOPK), np.uint32)
    for c in range(N_CORES):
        a = results[c]["cand"].reshape(128, NJ, TOPK)
        # target t = j*128 + p
        cand[c] = a.transpose(1, 0, 2).reshape(T, TOPK)
    return cand


# revision 17
# speedup vs baseline: 1.2764x; 1.0251x over previous
"""Trainium2 Bass kernel: greedy bbox-matching loss (nn_BboxLoss).

Full computation: L[t,p] = pairwise bbox loss (IoU / MSE mix), then greedy
per-target argmin over still-available preds, mean of selected losses.

Strategy (8 NeuronCores, preds sharded 8 x 1024, targets replicated):
  device: per core, for each 128-target row tile compute a rank key
          monotone in IoU for every (target, pred) pair and return the
          top-8 preds per target with the pred index PACKED into the low
          mantissa bits (one InstMax, no max_index):
          - corner overlap widths via tensor_scalar min/max (4x DVE mode)
            and PE identity-matmul accumulation into PSUM,
          - per-axis Ln on ACT: key = ln(m1)+ln(m2)-ln(area_p+area_t+eps)
            = ln(inter/S), monotone in IoU; non-overlap (m <= 0) becomes
            NaN which is clamped to -3e38 (junk) on Pool,
          - key assembled by f32r identity matmuls into PSUM, bitwise
            packed with a column iota on DVE, top-8 via one DVE max.
  host:   decode candidates, recompute exact reference-form losses for
          the 64 candidates per target, run the (inherently sequential)
          greedy walk with conservative full-row fallback when the
          candidate lists cannot prove the argmin.
"""
import numpy as np
from contextlib import ExitStack

P_TOTAL = 8192
T = 2048
N_CORES = 8
NP_SHARD = 8                  # pred shards (one per core)
P_CORE = P_TOTAL // NP_SHARD  # 1024 preds per core
NJ = T // 128                 # 16 row tiles of 128 targets
EPS = 1e-7
TOPK = 8
IDX_BITS = 10                 # P_CORE = 1024
IDX_MASK = (1 << IDX_BITS) - 1
KEY_MASK = 0xFFFFFFFF ^ IDX_MASK
DELTA = 0.02                  # corner clamp: forces widths >= ~DELTA (no NaN keys)
INVALID_THR = -1.0e28         # host-side validity threshold on decoded keys
MARGIN = 0.03                 # device-key approximation safety margin
RANK = 24                     # rank of the PE ln(areap+ate) approximation

_CACHE = {}


def _build_nc():
    import concourse.bacc as bacc
    import concourse.mybir as mybir
    from concourse.tile import TileContext

    f32 = mybir.dt.float32
    f32r = mybir.dt.float32r
    bf16 = mybir.dt.bfloat16
    u32 = mybir.dt.uint32
    Alu = mybir.AluOpType
    Act = mybir.ActivationFunctionType

    nc = bacc.Bacc()
    # x0p|x1p|y0p|y1p clipped corners, bf16
    pb_d = nc.dram_tensor("pshard", [1, 4 * P_CORE], bf16, kind="ExternalInput")
    tsc_d = nc.dram_tensor("tscal", [128, 9 * NJ], f32, kind="ExternalInput")
    idb_d = nc.dram_tensor("identb", [128, 256], bf16, kind="ExternalInput")
    idr_d = nc.dram_tensor("identr", [128, 256], f32r, kind="ExternalInput")
    iota_d = nc.dram_tensor("iotain", [1, P_CORE], u32, kind="ExternalInput")
    bf_d = nc.dram_tensor("basisF", [RANK, P_CORE], f32r, kind="ExternalInput")
    ng_d = nc.dram_tensor("negG", [RANK, NJ * 128], f32r, kind="ExternalInput")
    cand_d = nc.dram_tensor("cand", [128, NJ * TOPK], u32, kind="ExternalOutput")

    with TileContext(nc) as tc, ExitStack() as ctx:
        const = ctx.enter_context(tc.tile_pool(name="const", bufs=1))
        work = ctx.enter_context(tc.tile_pool(name="work", bufs=3))
        psA = ctx.enter_context(tc.tile_pool(name="psA", bufs=1, space="PSUM"))
        psB = ctx.enter_context(tc.tile_pool(name="psB", bufs=1, space="PSUM"))
        psF = ctx.enter_context(tc.tile_pool(name="psF", bufs=2, space="PSUM"))

        PL = const.tile([128, 4, P_CORE], bf16)   # corner planes
        TSC = const.tile([128, 9, NJ], f32)
        BF = const.tile([RANK, P_CORE], f32r)     # ln-approx pred basis
        NG = const.tile([RANK, NJ * 128], f32r)   # ln-approx -G coefs
        IDB = const.tile([128, 256], bf16)        # [I | -I] bf16
        IDR = const.tile([128, 256], f32r)        # [I | -I] f32r
        IOTA = const.tile([128, P_CORE], u32)
        MSKC = const.tile([128, 1], u32)
        CAND = const.tile([128, NJ, TOPK], u32)

        WU = const.tile([128, 1], f32)
        nc.vector.memset(MSKC[:], KEY_MASK)
        nc.vector.memset(WU[:], 1.0)
        # warm the Ln activation table while DMAs stream in
        nc.scalar.activation(WU[:], WU[:], Act.Ln)
        nc.sync.dma_start(TSC[:].rearrange("p q j -> p (q j)"), tsc_d[:])
        # replicate the per-pred rows across partitions; x planes first so
        # tile-0 corner work can start as early as possible
        PLF = PL[:].rearrange("p q n -> p (q n)")
        nc.sync.dma_start(
            PLF[:, 0 : 2 * P_CORE],
            pb_d[:, 0 : 2 * P_CORE].partition_broadcast(128),
        )
        nc.sync.dma_start(IDB[:], idb_d[:])
        nc.sync.dma_start(IDR[:], idr_d[:])
        nc.sync.dma_start(
            PLF[:, 2 * P_CORE :],
            pb_d[:, 2 * P_CORE :].partition_broadcast(128),
        )
        nc.sync.dma_start(IOTA[:], iota_d[:].partition_broadcast(128))
        nc.sync.dma_start(BF[:], bf_d[:])
        nc.sync.dma_start(NG[:], ng_d[:])

        X0P = PL[:, 0, :]
        X1P = PL[:, 1, :]
        Y0P = PL[:, 2, :]
        Y1P = PL[:, 3, :]
        ID_P = IDB[:, 0:128]
        ID_N = IDB[:, 128:256]
        IR_P = IDR[:, 0:128]
        IR_N = IDR[:, 128:256]
        NCH = P_CORE // 512

        def back_half(j, FIN, LNUV):
            PK = work.tile([128, P_CORE], u32, tag="pk")
            NGj = NG[:, j * 128 : (j + 1) * 128]
            for h in range(NCH):
                sl = slice(h * 512, (h + 1) * 512)
                sl2 = slice(P_CORE + h * 512, P_CORE + (h + 1) * 512)
                nc.tensor.matmul(FIN[:, sl], IR_P, LNUV[:, sl], start=True, stop=False)
                nc.tensor.matmul(FIN[:, sl], IR_P, LNUV[:, sl2], start=False, stop=False)
                nc.tensor.matmul(FIN[:, sl], NGj, BF[:, sl], start=False, stop=True)
            nc.vector.scalar_tensor_tensor(
                PK[:], FIN[:].bitcast(u32), MSKC[:, 0:1], IOTA[:],
                op0=Alu.bitwise_and, op1=Alu.bitwise_or,
            )
            nc.vector.max(CAND[:, j, :].bitcast(f32), PK[:].bitcast(f32))
            nc.sync.dma_start(
                cand_d[:, j * TOPK : (j + 1) * TOPK], CAND[:, j, :]
            )

        pending = None
        for j in range(NJ):
            x0t = TSC[:, 0, j : j + 1]
            x1t = TSC[:, 1, j : j + 1]
            y0t = TSC[:, 2, j : j + 1]
            y1t = TSC[:, 3, j : j + 1]
            ate = TSC[:, 4, j : j + 1]   # area_t + EPS
            x0d = TSC[:, 5, j : j + 1]   # x0t + DELTA
            x1d = TSC[:, 6, j : j + 1]   # x1t - DELTA
            y0d = TSC[:, 7, j : j + 1]
            y1d = TSC[:, 8, j : j + 1]

            CX = work.tile([128, P_CORE], bf16, tag="cx")
            MX = work.tile([128, P_CORE], bf16, tag="mx")
            CY = work.tile([128, P_CORE], bf16, tag="cy")
            MY = work.tile([128, P_CORE], bf16, tag="my")
            LNUV = work.tile([128, 2 * P_CORE], f32r, tag="lnuv")
            M1 = psA.tile([128, P_CORE], f32, tag="m1")
            M2 = psB.tile([128, P_CORE], f32, tag="m2")
            FIN = psF.tile([128, P_CORE], f32, tag="fin")

            # clamped corners: widths mx-cx / my-cy always >= ~DELTA > 0 so
            # Ln never produces NaN/-inf (which would poison the fin matmuls)
            nc.gpsimd.tensor_scalar(CX[:], X0P, x0t, x1d, op0=Alu.max, op1=Alu.min)
            nc.vector.tensor_scalar(MX[:], X1P, x1t, x0d, op0=Alu.min, op1=Alu.max)
            nc.gpsimd.tensor_scalar(CY[:], Y0P, y0t, y1d, op0=Alu.max, op1=Alu.min)
            nc.vector.tensor_scalar(MY[:], Y1P, y1t, y0d, op0=Alu.min, op1=Alu.max)

            # overlap widths in PSUM: m1 = mx - cx, m2 = my - cy
            for h in range(NCH):
                sl = slice(h * 512, (h + 1) * 512)
                nc.tensor.matmul(M1[:, sl], ID_P, MX[:, sl], start=True, stop=False)
                nc.tensor.matmul(M1[:, sl], ID_N, CX[:, sl], start=False, stop=True)
            for h in range(NCH):
                sl = slice(h * 512, (h + 1) * 512)
                nc.tensor.matmul(M2[:, sl], ID_P, MY[:, sl], start=True, stop=False)
                nc.tensor.matmul(M2[:, sl], ID_N, CY[:, sl], start=False, stop=True)

            # ln of (always positive) widths
            nc.scalar.activation(LNUV[:, 0:P_CORE], M1[:], Act.Ln)
            nc.scalar.activation(LNUV[:, P_CORE:], M2[:], Act.Ln)

            # deferred back half of the previous tile: fin matmuls after this
            # tile's m-matmuls keeps the PE stream bubble-free
            if pending is not None:
                back_half(*pending)
            pending = (j, FIN, LNUV)
        back_half(*pending)

    nc.compile()
    return nc


def _prep_core_inputs(pred, tgt):
    """Host-side O(P+T) derived quantities. pred [P,4], tgt [T,4] float32."""
    try:
        import ml_dtypes
        bf = ml_dtypes.bfloat16
    except Exception:
        import jax.numpy as jnp
        bf = jnp.bfloat16

    x0t = tgt[:, 0] - tgt[:, 2] / 2
    x1t = tgt[:, 0] + tgt[:, 2] / 2
    y0t = tgt[:, 1] - tgt[:, 3] / 2
    y1t = tgt[:, 1] + tgt[:, 3] / 2
    ate = tgt[:, 2] * tgt[:, 3] + np.float32(EPS)
    d = np.float32(DELTA)
    tscal = np.stack(
        [x0t, x1t, y0t, y1t, ate, x0t + d, x1t - d, y0t + d, y1t - d]
    ).astype(np.float32)  # [9, T]
    tsc = np.ascontiguousarray(
        tscal.reshape(9, NJ, 128).transpose(2, 0, 1).reshape(128, 9 * NJ)
    )

    iota = np.arange(P_CORE, dtype=np.uint32)[None, :]
    ident = np.eye(128, dtype=np.float32)
    idb = np.ascontiguousarray(
        np.concatenate([ident, -ident], axis=1)
    )
    idb_bf = idb.astype(bf)
    idr = idb.astype(np.float32)

    # rank-R separable approximation of ln(areap + ate):
    #   ln(x + y) ~ sum_k G[k, t] * F[k, p],  F = Chebyshev basis in ln(x+d0)
    # G fitted on a grid of y = ate values, then interpolated per target.
    d0 = np.float64(1e-7)
    umin, umax = np.log(d0), np.log(1.0 + 1e-3)
    bgrid = np.linspace(np.log(EPS * 0.99), np.log(1.0 + 1e-3), 384)
    ygrid = np.exp(bgrid)
    bt = np.log(ate.astype(np.float64))
    bt_c = np.clip(bt, bgrid[0], bgrid[-1])

    in_maps = []
    for c in range(N_CORES):
        sh = pred[c * P_CORE : (c + 1) * P_CORE]
        x0p = np.maximum(sh[:, 0] - sh[:, 2] / 2, np.float32(0.0))
        x1p = np.minimum(sh[:, 0] + sh[:, 2] / 2, np.float32(1.0))
        y0p = np.maximum(sh[:, 1] - sh[:, 3] / 2, np.float32(0.0))
        y1p = np.minimum(sh[:, 1] + sh[:, 3] / 2, np.float32(1.0))
        areap = (sh[:, 2] * sh[:, 3]).astype(np.float64)
        pshard = np.ascontiguousarray(
            np.stack([x0p, x1p, y0p, y1p]).astype(bf).reshape(1, 4 * P_CORE)
        )
        u = np.log(areap + d0)
        xnorm = np.clip((2.0 * u - (umin + umax)) / (umax - umin), -1.0, 1.0)
        A = np.polynomial.chebyshev.chebvander(xnorm, RANK - 1)      # [P, R]
        L = np.log(areap[:, None] + ygrid[None, :])                  # [P, G]
        Ggrid, *_ = np.linalg.lstsq(A, L, rcond=None)                # [R, G]
        # interpolate G columns at each target's b = ln(ate)
        pos = np.searchsorted(bgrid, bt_c, side="left").clip(1, len(bgrid) - 1)
        w = (bt_c - bgrid[pos - 1]) / (bgrid[pos] - bgrid[pos - 1])
        Gt = Ggrid[:, pos - 1] * (1.0 - w) + Ggrid[:, pos] * w       # [R, T]
        in_maps.append(
            {
                "pshard": pshard,
                "tscal": tsc,
                "identb": idb_bf,
                "identr": idr,
                "iotain": iota,
                "basisF": np.ascontiguousarray(A.T.astype(np.float32)),
                "negG": np.ascontiguousarray(
                    (-Gt.reshape(RANK, NJ, 128)).reshape(RANK, NJ * 128)
                    .astype(np.float32)
                ),
            }
        )
    return in_maps


def _pair_losses(p, t):
    """Reference-form loss for matched pairs p[i] <-> t[i] (numpy f32->f64)."""
    p = p.astype(np.float32); t = t.astype(np.float32)
    x0p = np.maximum(p[:, 0] - p[:, 2] / 2, np.float32(0.0))
    x1p = np.minimum(p[:, 0] + p[:, 2] / 2, np.float32(1.0))
    y0p = np.maximum(p[:, 1] - p[:, 3] / 2, np.float32(0.0))
    y1p = np.minimum(p[:, 1] + p[:, 3] / 2, np.float32(1.0))
    x0t = t[:, 0] - t[:, 2] / 2
    x1t = t[:, 0] + t[:, 2] / 2
    y0t = t[:, 1] - t[:, 3] / 2
    y1t = t[:, 1] + t[:, 3] / 2
    ox0 = np.maximum(x0t, x0p); ox1 = np.minimum(x1t, x1p)
    oy0 = np.maximum(y0t, y0p); oy1 = np.minimum(y1t, y1p)
    nov = (ox1 < ox0) | (oy1 < oy0)
    inter = (ox1 - ox0) * (oy1 - oy0)
    denom = p[:, 2] * p[:, 3] + t[:, 2] * t[:, 3] - inter + np.float32(EPS)
    iou = inter / denom
    mse = np.sum((p - t) * (p - t), axis=-1) / np.float32(4.0)
    return np.where(nov, np.float32(1.0) + mse,
                    np.float32(1.0) - iou).astype(np.float64)


def _row_loss_ref(pred, trow):
    """Reference-form loss of one target row vs all preds (numpy f32)."""
    x0p = np.maximum(pred[:, 0] - pred[:, 2] / 2, np.float32(0.0))
    x1p = np.minimum(pred[:, 0] + pred[:, 2] / 2, np.float32(1.0))
    y0p = np.maximum(pred[:, 1] - pred[:, 3] / 2, np.float32(0.0))
    y1p = np.minimum(pred[:, 1] + pred[:, 3] / 2, np.float32(1.0))
    x0t = trow[0] - trow[2] / 2
    x1t = trow[0] + trow[2] / 2
    y0t = trow[1] - trow[3] / 2
    y1t = trow[1] + trow[3] / 2
    ox0 = np.maximum(x0t, x0p); ox1 = np.minimum(x1t, x1p)
    oy0 = np.maximum(y0t, y0p); oy1 = np.minimum(y1t, y1p)
    nov = (ox1 < ox0) | (oy1 < oy0)
    inter = (ox1 - ox0) * (oy1 - oy0)
    denom = pred[:, 2] * pred[:, 3] + trow[2] * trow[3] - inter + np.float32(EPS)
    iou = inter / denom
    d = pred - trow[None, :]
    mse = np.sum(d * d, axis=-1) / np.float32(4.0)
    return np.where(nov, np.float32(1.0) + mse, np.float32(1.0) - iou)


def _host_greedy(cand_u32, pred, tgt):
    """cand_u32 [N_CORES, T, TOPK]: packed top-8 per (target, pred shard)."""
    NSH = N_CORES
    u = cand_u32.transpose(1, 0, 2).reshape(T, NSH * TOPK)   # [T, 64]
    idx_l = (u & np.uint32(IDX_MASK)).astype(np.int64)
    shard_of = np.broadcast_to(
        np.arange(NSH, dtype=np.int64)[None, :, None], (T, NSH, TOPK)
    ).reshape(T, NSH * TOPK)
    gidx = shard_of * P_CORE + idx_l
    keyf = (u & np.uint32(KEY_MASK)).view(np.float32)
    valid = np.isfinite(keyf) & (keyf > INVALID_THR)

    # exact reference-form loss for every candidate
    tgt_rep = np.repeat(tgt, NSH * TOPK, axis=0)
    loss = _pair_losses(pred[gidx.reshape(-1)], tgt_rep).reshape(T, NSH * TOPK)
    loss[~valid] = np.inf

    order = np.lexsort((gidx, loss), axis=1)                 # per-row

    # sound hidden-candidate bound per (row, shard): every unlisted pair has
    # (possibly delta-boosted) device key <= the 8th listed key, and boosting
    # only raises keys, so its true iou <= iou(key8) and its true loss
    # >= 1 - iou(key8).  key8 decodes >= the stored key (mask clears low
    # mantissa bits of a negative float), keeping the bound conservative.
    key8 = keyf.reshape(T, NSH, TOPK)[:, :, TOPK - 1].astype(np.float64)
    g8 = np.exp(np.minimum(key8, -1e-12))
    hidden_bound = 1.0 - g8 / (1.0 - g8)                     # [T, NSH]
    hidden_bound_min = hidden_bound.min(axis=1)              # [T]

    taken = np.zeros(P_TOTAL, dtype=bool)
    sel = np.empty(T, dtype=np.int64)
    n_fallback = 0
    for t in range(T):
        lt = loss[t]; gt = gidx[t]; ot = order[t]
        chosen = -1
        for d in ot:
            if lt[d] == np.inf:
                break
            if not taken[gt[d]]:
                chosen = d
                break
        safe = chosen >= 0
        if safe:
            closs = lt[chosen]
            # hidden mse-branch pairs have loss >= 1; hidden overlap pairs
            # are bounded by the per-shard key8 bound
            if closs >= np.float32(1.0) - MARGIN:
                safe = False
            elif hidden_bound_min[t] < closs + MARGIN:
                safe = False
        if safe:
            k = gt[chosen]
        else:
            n_fallback += 1
            row = _row_loss_ref(pred, tgt[t]).astype(np.float64)
            row[taken] = np.inf
            k = int(np.argmin(row))
        taken[k] = True
        sel[t] = k
    _host_greedy.n_fallback = n_fallback
    return np.float32(_pair_losses(pred[sel], tgt).mean())


def kernel(pred_bboxes, target_bboxes):
    from concourse.bass_utils import run_bass_kernel_spmd

    pred = np.asarray(pred_bboxes, dtype=np.float32)[0]
    tgt = np.asarray(target_bboxes, dtype=np.float32)[0]

    if "nc" not in _CACHE:
        _CACHE["nc"] = _build_nc()
    nc = _CACHE["nc"]

    in_maps = _prep_core_inputs(pred, tgt)
    res = run_bass_kernel_spmd(nc, in_maps, list(range(N_CORES)))
    cand = _collect(res.results)
    return _host_greedy(cand, pred, tgt)


def _collect(results):
    """results[c]['cand'] [128, NJ*TOPK] u32 -> [N_CORES, T, TOPK]."""
    cand = np.empty((N_CORES, T, TOPK), np.uint32)
    for c in range(N_CORES):
        a = results[c]["cand"].reshape(128, NJ, TOPK)
        # target t = j*128 + p
        cand[c] = a.transpose(1, 0, 2).reshape(T, TOPK)
    return cand
